# revision 3
# baseline (speedup 1.0000x reference)
"""ClashLoss kernel for Trainium2 (8 NeuronCores, batch-parallel).

Math: for each batch b, count pairs (n, m) with
    dist(n, m) < r_n + r_m   and   dist(n, m) > EPS.
Equivalent test (squared form, h = (|c|^2 - r^2)/2):
    x_n x_m + y_n y_m + z_n z_m + r_n r_m - h_n  >  h_m .

The execution backend charges a large, nearly size-independent cost per
engine INSTRUCTION, so the kernel minimizes instruction count rather
than classical FLOP/byte rooflines.  The pair matrix is evaluated on the
DVE engine in 4 "giant chains", one per 1024-row group g, restricted to
the upper-triangular column window [1024 g, 4096).  Each chain covers
T=8 row-blocks in 10 instructions operating on [128, 8, w] access
patterns (f16 work buffers, up to 64 KB/partition):

    A  = Ux (x) Vx          broadcast outer-product  (tensor_tensor mult)
    P  = Uy (x) Vy ; A += P
    P  = Uz (x) Vz ; A += P
    P  = Ur (x) Vr ; A += P
    A -= H                  (h_n, broadcast along columns)
    cnt[2g]   = sum 1[A > W] over the group-diagonal 1024 columns
    cnt[2g+1] = sum 1[A > W] over the strictly-right columns

U-side operands are per-(partition, row-block) scalars broadcast along
columns via stride-0 AP dims; V-side operands are row-replicated vectors
broadcast across row-blocks.  The diagonal 1024x1024 block of group g
counts each same-group ordered pair (both orders) plus the n == m
diagonal (always positive: margin 2 r_n^2 > 0); the right part counts
each cross-group unordered pair exactly once.  Host combine:
    num_clashes_b = sum(right) + (sum(diag) - N) / 2
    loss = mean_b num_clashes_b / N .
f16 work buffers flip only pairs whose clash margin is below ~1e-3 of
scale; measured end-to-end rel err vs the f32 reference is ~1.3e-3
(gate: 2e-2).
"""

import numpy as np

N = 4096
B = 8
T = 8            # row-blocks per chain
NGROUP = 4       # 1024-row groups
_CACHE = {}


def _build(repeat=1):
    import concourse.bass as bass
    from concourse import mybir

    nc = bass.Bass("TRN2", target_bir_lowering=False, debug=False)
    f16 = mybir.dt.float16
    f32 = mybir.dt.float32

    vpack_d = nc.dram_tensor("vpack", [128, 5 * N], f16, kind="ExternalInput").ap()
    upack_d = nc.dram_tensor("upack", [128, 5 * 32], f16, kind="ExternalInput").ap()
    cnt_d = nc.dram_tensor("cnt", [128, 2 * NGROUP], f32, kind="ExternalOutput").ap()

    with (
        nc.sbuf_tensor([128, 5 * N], f16) as vpack,
        nc.sbuf_tensor([128, 5 * 32], f16) as upack,
        nc.sbuf_tensor([128, T * N], f16) as A,
        nc.sbuf_tensor([128, T * N], f16) as P,
        nc.sbuf_tensor([128, 2 * NGROUP], f32) as cnt,
        nc.semaphore("SIN") as s_in,
        nc.semaphore("SDONE") as s_done,
        nc.semaphore("SOUT") as s_out,
        nc.Block() as block,
    ):
        # vpack columns: [x | y | z | r | h], each N wide, row-replicated.
        # upack columns: [x | y | z | r | h], each 32 wide; upack[p, k*32+t]
        # is component k of atom 128*t + p.
        def vap(k, c0, w):
            lo = k * N + c0
            return vpack.ap()[:, lo:lo + w].unsqueeze(1).broadcast_to((128, T, w))

        def uap(k, b0, w):
            lo = k * 32 + b0
            return upack.ap()[:, lo:lo + T].unsqueeze(2).broadcast_to((128, T, w))

        @block.sync
        def _(sync):
            sync.dma_start(out=vpack[:, :], in_=vpack_d).then_inc(s_in, 16)
            sync.dma_start(out=upack[:, :], in_=upack_d).then_inc(s_in, 16)
            sync.wait_ge(s_done, repeat)
            sync.dma_start(out=cnt_d, in_=cnt[:, :]).then_inc(s_out, 16)
            sync.wait_ge(s_out, 16)

        @block.vector
        def _(vector):
            vector.wait_ge(s_in, 32)
            mult = mybir.AluOpType.mult
            add = mybir.AluOpType.add
            sub = mybir.AluOpType.subtract
            is_gt = mybir.AluOpType.is_gt
            for rep in range(repeat):
                for g in range(NGROUP):
                    c0 = 1024 * g       # column window start
                    w = N - c0          # column window width
                    b0 = T * g          # first row-block of the group
                    o = A.ap()[:, 0:T * w].rearrange("p (t j) -> p t j", t=T)
                    po = P.ap()[:, 0:T * w].rearrange("p (t j) -> p t j", t=T)
                    nc.vector.tensor_tensor(out=o, in0=uap(0, b0, w), in1=vap(0, c0, w), op=mult)
                    nc.vector.tensor_tensor(out=po, in0=uap(1, b0, w), in1=vap(1, c0, w), op=mult)
                    nc.vector.tensor_tensor(out=o, in0=o, in1=po, op=add)
                    nc.vector.tensor_tensor(out=po, in0=uap(2, b0, w), in1=vap(2, c0, w), op=mult)
                    nc.vector.tensor_tensor(out=o, in0=o, in1=po, op=add)
                    nc.vector.tensor_tensor(out=po, in0=uap(3, b0, w), in1=vap(3, c0, w), op=mult)
                    nc.vector.tensor_tensor(out=o, in0=o, in1=po, op=add)
                    nc.vector.tensor_tensor(out=o, in0=o, in1=uap(4, b0, w), op=sub)
                    od = o[:, :, 0:1024]
                    ins = nc.vector.scalar_tensor_tensor(
                        out=od, in0=od, scalar=0.0, in1=vap(4, c0, 1024),
                        op0=add, op1=is_gt, accum_out=cnt[:, 2 * g:2 * g + 1])
                    if w > 1024:
                        orr = o[:, :, 1024:w]
                        ins = nc.vector.scalar_tensor_tensor(
                            out=orr, in0=orr, scalar=0.0, in1=vap(4, c0 + 1024, w - 1024),
                            op0=add, op1=is_gt, accum_out=cnt[:, 2 * g + 1:2 * g + 2])
                    if g == NGROUP - 1:
                        ins.then_inc(s_done, 1)
    return nc


def _prep_inputs(coords, atom_types, vdw_radii):
    """Host-side shard prep: per-batch f16 vpack/upack arrays."""
    coords = np.asarray(coords, dtype=np.float32)   # [B, N, 3]
    atom_types = np.asarray(atom_types).astype(np.int64)
    vdw = np.asarray(vdw_radii, dtype=np.float32)
    r = vdw[atom_types]                              # [B, N]
    sq = np.einsum("bnd,bnd->bn", coords, coords).astype(np.float32)
    h = ((sq - r * r) / 2.0).astype(np.float32)
    in_maps = []
    for b in range(coords.shape[0]):
        x, y, z = coords[b, :, 0], coords[b, :, 1], coords[b, :, 2]
        comps = (x, y, z, r[b], h[b])
        vp = np.empty((128, 5 * N), np.float16)
        up = np.empty((128, 5 * 32), np.float16)
        for k, arr in enumerate(comps):
            a16 = arr.astype(np.float16)
            vp[:, k * N:(k + 1) * N] = a16[None, :]
            up[:, k * 32:(k + 1) * 32] = a16.reshape(32, 128).T
        in_maps.append({"vpack": vp, "upack": up})
    return in_maps


def _combine(results):
    """Host-side gather: per-core count slots -> scalar loss."""
    total = 0.0
    for b in range(len(results)):
        c = np.asarray(results[b]["cnt"], np.float64)
        diag = c[:, 0::2].sum()    # same-group ordered pairs + self-pairs
        right = c[:, 1::2].sum()   # cross-group unordered pairs, once each
        total += (right + (diag - N) / 2.0) / N
    return np.float32(total / len(results))


def kernel(coords, atom_types, vdw_radii):
    import sys

    if "/opt/trn_rl_repo" not in sys.path:
        sys.path.insert(0, "/opt/trn_rl_repo")
    from concourse.bass_utils import run_bass_kernel_spmd

    if "nc" not in _CACHE:
        _CACHE["nc"] = _build()
    nc = _CACHE["nc"]

    in_maps = _prep_inputs(coords, atom_types, vdw_radii)
    res = run_bass_kernel_spmd(nc, in_maps, core_ids=list(range(B)))
    return _combine(res.results)


if __name__ == "__main__":
    import sys

    sys.path.insert(0, "/root/problem")
    import reference as ref

    inputs = ref.setup_inputs()
    out = kernel(**{k: np.asarray(v) for k, v in inputs.items()})
    print("kernel output:", out)
